# revision 3
# baseline (speedup 1.0000x reference)
"""Cached scaled-dot-product-attention decode kernel for Trainium2 (Bass/Tile).

Full inputs -> shard batch across 8 NeuronCores (B=8, one batch per core)
-> per-core Bass kernel computes, for each of its 32 heads:
    K = cache_k[h] with row cache_pos replaced by key[h]
    V = cache_v[h] with row cache_pos replaced by value[h]
    out[h] = softmax(q K^T / sqrt(D)) V        (over the first cache_pos+1 rows)
-> gather per-core outputs into the full [B, H, 1, D] array.

Layout trick: cache_k[h] ([S, D] row-major in HBM) is loaded as SBUF
[128, S] via "(p r) d -> p (r d)" so every partition reads one fully
contiguous 16KB chunk (max DMA efficiency).  Sequence position
s = p*R + r lands at (partition p, column-block r).  This is a fixed
permutation of the sequence axis, which softmax(..)V is invariant to, as
long as K and V use the same permutation (they do).

The kernel is HBM-bandwidth-bound (128 MiB of cache per core vs the
~358 GB/s HBM-per-NeuronCore limit -> ~375 us floor), so the entire
design keeps the two DMA streams (K on the sync/SP HWDGE ring, V on the
gpsimd SWDGE ring) running back-to-back with zero completion-coupled
stalls:

 - No cache-row scatter DMAs.  The decode-step key/value are NOT written
   into the loaded tiles (which would serialize each queue on the
   previous load's completion receipt).  Instead the stale cache row's
   contribution is removed and the new row's added algebraically in the
   PSUM accumulation:
       out_unnorm += p_new * value[h] - p_stale * V_cache[pos]
       Z          += p_new           - p_stale
   via two extra rank-1 matmuls per head (one-hot masked coefficients),
   where p_stale = exp(q . K_cache[pos] * scale) falls out of the normal
   score pipeline and p_new = exp(q . key[h] * scale) is precomputed for
   all heads in the prologue.
 - kv tile pools are 4-deep so a load's WAR dependency (4 heads back) is
   always long resolved; load triggers never gate the descriptor rings.
 - DVE runs only tensor_tensor / tensor_reduce (+ a [1,1] reciprocal),
   which never contend for the shared SBUF port pair with GpSimd's SWDGE
   descriptor generation; per-head epilogue scaling runs on ACT.
 - The last head is split 4-ways (K load, mult, reduce, exp, attn@V all
   chunked) so the post-last-byte drain is a quarter-chain, and the
   output for heads 0..30 is written out early.

Scores are computed on the DVE (one big elementwise multiply against a
partition-broadcast q, then a 3D tensor_reduce over d) so K never needs
a transpose.  attn@V contracts over the partition axis on the PE
(lhsT = prob column, rhs = natural V tile); softmax normalization is a
single reciprocal + ACT scale at the end (exp is unshifted — scores
are ~N(0,1) so fp32 exp cannot overflow).
"""

import math
from contextlib import ExitStack

import numpy as np

import concourse.bacc as bacc
import concourse.mybir as mybir
import concourse.tile as tile
from concourse.bass_utils import run_bass_kernel_spmd

F32 = mybir.dt.float32

N_CORES = 8

_program_cache: dict = {}
_last_results = None


def _build(H: int, S: int, D: int, cache_pos: int):
    """Build + compile the per-core Bass program (identical on all cores)."""
    P = 128
    R = S // P  # column blocks / rows-per-partition (32 for S=4096)
    assert S % P == 0 and D == 128
    end_pos = cache_pos + 1
    scale = 1.0 / math.sqrt(D)

    nc = bacc.Bacc(
        "TRN2",
        target_bir_lowering=False,
        debug=False,
        enable_asserts=False,
        num_devices=N_CORES,
    )
    q_d = nc.dram_tensor("query", [H, 1, D], F32, kind="ExternalInput").ap()
    k_d = nc.dram_tensor("key", [H, 1, D], F32, kind="ExternalInput").ap()
    v_d = nc.dram_tensor("value", [H, 1, D], F32, kind="ExternalInput").ap()
    ck_d = nc.dram_tensor("cache_k", [H, S, D], F32, kind="ExternalInput").ap()
    cv_d = nc.dram_tensor("cache_v", [H, S, D], F32, kind="ExternalInput").ap()
    out_d = nc.dram_tensor("out", [1, H * D], F32, kind="ExternalOutput").ap()

    pp = cache_pos // R  # partition holding the patched row
    rr = cache_pos % R  # column block holding the patched row

    with tile.TileContext(nc) as tc, ExitStack() as ctx:
        const_pool = ctx.enter_context(tc.tile_pool(name="const", bufs=1))
        kv_pool = ctx.enter_context(tc.tile_pool(name="kv", bufs=4))
        sm_pool = ctx.enter_context(tc.tile_pool(name="sm", bufs=2))
        ps_build = ctx.enter_context(tc.tile_pool(name="psb", bufs=2, space="PSUM"))
        ps_av = ctx.enter_context(tc.tile_pool(name="psav", bufs=2, space="PSUM"))
        ps_z = ctx.enter_context(tc.tile_pool(name="psz", bufs=2, space="PSUM"))

        # ---- head 0's big loads go first so both DMA rings start instantly
        k_t0 = kv_pool.tile([P, S], F32, name="k_t", tag="k")
        nc.sync.dma_start(k_t0[:], ck_d[0].rearrange("(p r) d -> p (r d)", p=P))
        v_t0 = kv_pool.tile([P, S], F32, name="v_t", tag="v")
        nc.gpsimd.dma_start(v_t0[:], cv_d[0].rearrange("(p r) d -> p (r d)", p=P))

        # ---- prologue: constants + decode-row (key/value) correction terms
        ones_t = const_pool.tile([P, P], F32, name="ones_t")
        nc.vector.memset(ones_t[:], 1.0)
        ones_row = ones_t[0:1, :]
        ones_col = ones_t[:, 0:1]

        # -1 at partition pp, 0 elsewhere: masks out the stale cache row.
        piota = const_pool.tile([P, 1], F32, name="piota")
        nc.gpsimd.iota(
            piota[:], [[0, 1]], channel_multiplier=1,
            allow_small_or_imprecise_dtypes=True,
        )
        neg_e_pp = const_pool.tile([P, 1], F32, name="neg_e_pp")
        nc.vector.tensor_scalar(
            neg_e_pp[:],
            piota[:],
            float(pp),
            -1.0,
            op0=mybir.AluOpType.is_equal,
            op1=mybir.AluOpType.mult,
        )

        # q / key / value as [H, D] tiles (partition = head) on the ACT ring.
        q32 = const_pool.tile([H, D], F32, name="q32")
        nc.scalar.dma_start(q32[:], q_d.rearrange("h q d -> (h q) d"))
        key32 = const_pool.tile([H, D], F32, name="key32")
        nc.scalar.dma_start(key32[:], k_d.rearrange("h q d -> (h q) d"))
        value32 = const_pool.tile([H, D], F32, name="value32")
        nc.scalar.dma_start(value32[:], v_d.rearrange("h q d -> (h q) d"))

        # p_new[h] = exp(q_h . key_h * scale); coefs[:, h] = p_new[h] * e_h
        qk_prod = const_pool.tile([H, D], F32, name="qk_prod")
        nc.vector.tensor_tensor(qk_prod[:], q32[:], key32[:], op=mybir.AluOpType.mult)
        s_new = const_pool.tile([H, 1], F32, name="s_new")
        nc.vector.tensor_reduce(
            s_new[:], qk_prod[:], axis=mybir.AxisListType.X, op=mybir.AluOpType.add
        )
        p_new = const_pool.tile([H, 1], F32, name="p_new")
        nc.scalar.activation(
            p_new[:], s_new[:], mybir.ActivationFunctionType.Exp, scale=float(scale)
        )
        hmat = const_pool.tile([H, H], F32, name="hmat")
        nc.gpsimd.iota(
            hmat[:],
            [[-1, H]],
            channel_multiplier=1,
            allow_small_or_imprecise_dtypes=True,
        )
        coefs = const_pool.tile([H, H], F32, name="coefs")
        nc.vector.tensor_scalar(
            coefs[:],
            hmat[:],
            0.0,
            p_new[:],
            op0=mybir.AluOpType.is_equal,
            op1=mybir.AluOpType.mult,
        )

        out_stage = const_pool.tile([1, H * D], F32, name="out_stage")
        # out_stage doubles as the q staging row during the prologue (it is
        # only written by the per-head epilogues, which depend on q_bc).
        q_flat = out_stage
        nc.scalar.dma_start(q_flat[:], q_d.rearrange("h q d -> q (h d)"))
        q_bc = const_pool.tile([P, H * D], F32, name="q_bc")
        NB = 512
        for j in range((H * D + NB - 1) // NB):
            nb = min(NB, H * D - j * NB)
            qb_ps = ps_build.tile([P, NB], F32, name="qb_ps")
            nc.tensor.matmul(
                qb_ps[:, :nb],
                ones_row[:],
                q_flat[0:1, j * NB : j * NB + nb],
                start=True,
                stop=True,
            )
            # fold the 1/sqrt(D) softmax scale into the broadcast copy
            nc.scalar.mul(q_bc[:, j * NB : j * NB + nb], qb_ps[:, :nb], scale)

        mask = None
        if end_pos < S:
            # Additive score mask: 0 where s = p*R + r < end_pos, -1e30 after.
            s_iota = const_pool.tile([P, R], F32, name="s_iota")
            nc.gpsimd.iota(
                s_iota[:],
                [[1, R]],
                channel_multiplier=R,
                allow_small_or_imprecise_dtypes=True,
            )
            mask = const_pool.tile([P, R], F32, name="mask")
            nc.vector.tensor_scalar(
                mask[:],
                s_iota[:],
                float(end_pos),
                -1e30,
                op0=mybir.AluOpType.is_ge,
                op1=mybir.AluOpType.mult,
            )

        for h in range(H):
            # The last head's chain (mult -> reduce -> exp -> attn@V) is the
            # kernel's drain tail: split its stages in quarters so each stage
            # overlaps the tail of its K load and the chain after the last
            # HBM byte is only a quarter-chain.  Other heads stay whole
            # (splitting every load costs DMA descriptor efficiency).
            last = h == H - 1
            nsplit = 4 if last else 1
            RC, SC = R // nsplit, S // nsplit

            # All-fp32 numerics. K loads ride the HWDGE (sync) ring, V loads
            # the SWDGE (gpsimd) ring — splitting the two 2MiB streams across
            # both descriptor-generation paths keeps the SDMA engines fed.
            if h == 0:
                k_t, v_t = k_t0, v_t0
            else:
                k_t = kv_pool.tile([P, S], F32, name="k_t", tag="k")
                ck_h = ck_d[h].rearrange("(p r) d -> p (r d)", p=P)
                for c in range(nsplit):
                    nc.sync.dma_start(
                        k_t[:, c * SC : (c + 1) * SC], ck_h[:, c * SC : (c + 1) * SC]
                    )
                v_t = kv_pool.tile([P, S], F32, name="v_t", tag="v")
                cv_h = cv_d[h].rearrange("(p r) d -> p (r d)", p=P)
                vsplit = 2 if last else 1
                VC = S // vsplit
                for c in range(vsplit):
                    nc.gpsimd.dma_start(
                        v_t[:, c * VC : (c + 1) * VC], cv_h[:, c * VC : (c + 1) * VC]
                    )

            # scores[p, r] = sum_d K[p, r, d] * q_scaled[d]   for s = p*R + r
            scores = sm_pool.tile([P, R], F32, name="scores", tag="scores")
            prod = sm_pool.tile([P, S], F32, name="prod", tag="prod", bufs=1)
            p_t = sm_pool.tile([P, R], F32, name="p_t", tag="p")
            z_cols = []
            for c in range(nsplit):
                qh = (
                    q_bc[:, h * D : (h + 1) * D]
                    .rearrange("p (o d) -> p o d", o=1)
                    .broadcast_to([P, RC, D])
                )
                k3 = k_t[:, c * SC : (c + 1) * SC].rearrange("p (r d) -> p r d", r=RC)
                prod3 = prod[:, c * SC : (c + 1) * SC].rearrange(
                    "p (r d) -> p r d", r=RC
                )
                sc_c = scores[:, c * RC : (c + 1) * RC]
                nc.vector.tensor_tensor(prod3, k3, qh, op=mybir.AluOpType.mult)
                nc.vector.tensor_reduce(
                    sc_c, prod3, axis=mybir.AxisListType.X, op=mybir.AluOpType.add
                )
                if mask is not None:
                    nc.vector.tensor_tensor(
                        sc_c,
                        sc_c,
                        mask[:, c * RC : (c + 1) * RC],
                        op=mybir.AluOpType.add,
                    )
                # p = exp(scores); z_col[p] = partial softmax denominator
                z_col = sm_pool.tile([P, 1], F32, name="z_col", tag=f"z{c}")
                nc.scalar.activation(
                    p_t[:, c * RC : (c + 1) * RC],
                    sc_c,
                    mybir.ActivationFunctionType.Exp,
                    accum_out=z_col[:],
                )
                z_cols.append(z_col)

            # tmp = -p_stale at partition pp (0 elsewhere): removes the stale
            # cache row's contribution from both attn@V and Z.
            tmp = sm_pool.tile([P, 1], F32, name="tmp", tag="tmp")
            nc.vector.tensor_tensor(
                tmp[:], p_t[:, rr : rr + 1], neg_e_pp[:], op=mybir.AluOpType.mult
            )

            # out_unnorm[1, D] = sum_r p[:, r]^T @ V_tile_r  (+ corrections)
            av_ps = ps_av.tile([1, D], F32, name="av_ps")
            for r in range(R):
                nc.tensor.matmul(
                    av_ps[:],
                    p_t[:, r : r + 1],
                    v_t[:, r * D : (r + 1) * D],
                    start=(r == 0),
                    stop=False,
                )
            nc.tensor.matmul(
                av_ps[:],
                tmp[:],
                v_t[:, rr * D : (rr + 1) * D],
                start=False,
                stop=False,
            )
            nc.tensor.matmul(
                av_ps[:],
                coefs[:, h : h + 1],
                value32[:],
                start=False,
                stop=True,
            )
            # Z = sum over partitions of the z_col partials (+ corrections)
            z_ps = ps_z.tile([1, 1], F32, name="z_ps")
            for c, z_col in enumerate(z_cols):
                nc.tensor.matmul(
                    z_ps[:], z_col[:], ones_col[:], start=(c == 0), stop=False
                )
            nc.tensor.matmul(z_ps[:], tmp[:], ones_col[:], start=False, stop=False)
            nc.tensor.matmul(
                z_ps[:], coefs[:, h : h + 1], ones_col[0:H, :], start=False, stop=True
            )
            rz = sm_pool.tile([1, 1], F32, name="rz", tag="rz")
            nc.vector.reciprocal(rz[:], z_ps[:])
            nc.scalar.mul(out_stage[0:1, h * D : (h + 1) * D], av_ps[:], rz[:])
            if h == H - 2:
                # everything but the last head is final: ship it early so the
                # drain only carries the last head's 512B.
                nc.scalar.dma_start(
                    out_d[0:1, : (H - 1) * D], out_stage[0:1, : (H - 1) * D]
                )

        nc.scalar.dma_start(
            out_d[0:1, (H - 1) * D :], out_stage[0:1, (H - 1) * D :]
        )

    nc.compile()
    return nc


def _get_program(H, S, D, cache_pos):
    key = (H, S, D, cache_pos)
    if key not in _program_cache:
        _program_cache[key] = _build(H, S, D, cache_pos)
    return _program_cache[key]


def kernel(query, key, value, cache_k, cache_v, cache_pos):
    cache_pos = int(cache_pos)
    B, H, Q, D = query.shape
    S = cache_k.shape[2]
    assert Q == 1 and B == N_CORES

    nc = _get_program(H, S, D, cache_pos)

    f32 = np.float32
    in_maps = [
        {
            "query": np.ascontiguousarray(query[b], dtype=f32),
            "key": np.ascontiguousarray(key[b], dtype=f32),
            "value": np.ascontiguousarray(value[b], dtype=f32),
            "cache_k": np.ascontiguousarray(cache_k[b], dtype=f32),
            "cache_v": np.ascontiguousarray(cache_v[b], dtype=f32),
        }
        for b in range(B)
    ]
    res = run_bass_kernel_spmd(nc, in_maps, core_ids=list(range(N_CORES)))
    global _last_results
    _last_results = res
    out = np.stack(
        [res.results[b]["out"].reshape(H, 1, D).astype(np.float32) for b in range(B)]
    )
    return out


# revision 5
# speedup vs baseline: 1.1610x; 1.1610x over previous
"""Cached scaled-dot-product-attention decode kernel for Trainium2 (Bass/Tile).

Full inputs -> shard batch across 8 NeuronCores (B=8, one batch per core)
-> per-core Bass kernel computes, for each of its 32 heads:
    K = cache_k[h] with row cache_pos replaced by key[h]
    V = cache_v[h] with row cache_pos replaced by value[h]
    out[h] = softmax(q K^T / sqrt(D)) V        (over the first cache_pos+1 rows)
-> gather per-core outputs into the full [B, H, 1, D] array.

Layout trick: cache_k[h] ([S, D] row-major in HBM) is loaded as SBUF
[128, S] via "(p r) d -> p (r d)" so every partition reads one fully
contiguous 16KB chunk (max DMA efficiency).  Sequence position
s = p*R + r lands at (partition p, column-block r).  This is a fixed
permutation of the sequence axis, which softmax(..)V is invariant to, as
long as K and V use the same permutation (they do).

The kernel is HBM-bandwidth-bound (128 MiB of cache per core vs the
~358 GB/s HBM-per-NeuronCore limit -> ~375 us floor), so the entire
design keeps the two DMA streams (K on the sync/SP HWDGE ring, V on the
gpsimd SWDGE ring) running back-to-back with zero completion-coupled
stalls:

 - No cache-row scatter DMAs.  The decode-step key/value are NOT written
   into the loaded tiles (which would serialize each queue on the
   previous load's completion receipt).  Instead the stale cache row's
   contribution is removed and the new row's added algebraically in the
   PSUM accumulation:
       out_unnorm += p_new * value[h] - p_stale * V_cache[pos]
       Z          += p_new           - p_stale
   via two extra rank-1 matmuls per head (one-hot masked coefficients),
   where p_stale = exp(q . K_cache[pos] * scale) falls out of the normal
   score pipeline and p_new = exp(q . key[h] * scale) is precomputed for
   all heads in the prologue.
 - kv tile pools are 4-deep so a load's WAR dependency (4 heads back) is
   always long resolved; load triggers never gate the descriptor rings.
 - DVE runs only tensor_tensor / tensor_reduce (+ a [1,1] reciprocal),
   which never contend for the shared SBUF port pair with GpSimd's SWDGE
   descriptor generation; per-head epilogue scaling runs on ACT.
 - The last head is split 4-ways (K load, mult, reduce, exp, attn@V all
   chunked) so the post-last-byte drain is a quarter-chain, and the
   output for heads 0..30 is written out early.

Scores are computed on the DVE (one big elementwise multiply against a
partition-broadcast q, then a 3D tensor_reduce over d) so K never needs
a transpose.  attn@V contracts over the partition axis on the PE
(lhsT = prob column, rhs = natural V tile); softmax normalization is a
single reciprocal + ACT scale at the end (exp is unshifted — scores
are ~N(0,1) so fp32 exp cannot overflow).
"""

import math
from contextlib import ExitStack

import numpy as np

import concourse.bacc as bacc
import concourse.mybir as mybir
import concourse.tile as tile
from concourse.bass_utils import run_bass_kernel_spmd

F32 = mybir.dt.float32

N_CORES = 8

_program_cache: dict = {}
_last_results = None


def _build(H: int, S: int, D: int, cache_pos: int):
    """Build + compile the per-core Bass program (identical on all cores)."""
    P = 128
    R = S // P  # column blocks / rows-per-partition (32 for S=4096)
    assert S % P == 0 and D == 128
    end_pos = cache_pos + 1
    scale = 1.0 / math.sqrt(D)

    nc = bacc.Bacc(
        "TRN2",
        target_bir_lowering=False,
        debug=False,
        enable_asserts=False,
        num_devices=N_CORES,
    )
    q_d = nc.dram_tensor("query", [H, 1, D], F32, kind="ExternalInput").ap()
    k_d = nc.dram_tensor("key", [H, 1, D], F32, kind="ExternalInput").ap()
    v_d = nc.dram_tensor("value", [H, 1, D], F32, kind="ExternalInput").ap()
    ck_d = nc.dram_tensor("cache_k", [H, S, D], F32, kind="ExternalInput").ap()
    cv_d = nc.dram_tensor("cache_v", [H, S, D], F32, kind="ExternalInput").ap()
    out_d = nc.dram_tensor("out", [1, H * D], F32, kind="ExternalOutput").ap()

    pp = cache_pos // R  # partition holding the patched row
    rr = cache_pos % R  # column block holding the patched row

    with tile.TileContext(nc) as tc, ExitStack() as ctx:
        const_pool = ctx.enter_context(tc.tile_pool(name="const", bufs=1))
        kv_pool = ctx.enter_context(tc.tile_pool(name="kv", bufs=4))
        sm_pool = ctx.enter_context(tc.tile_pool(name="sm", bufs=2))
        ps_build = ctx.enter_context(tc.tile_pool(name="psb", bufs=2, space="PSUM"))
        ps_av = ctx.enter_context(tc.tile_pool(name="psav", bufs=2, space="PSUM"))
        ps_z = ctx.enter_context(tc.tile_pool(name="psz", bufs=2, space="PSUM"))

        # ---- head 0's big loads go first so both DMA rings start instantly
        k_t0 = kv_pool.tile([P, S], F32, name="k_t", tag="k")
        nc.sync.dma_start(k_t0[:], ck_d[0].rearrange("(p r) d -> p (r d)", p=P))
        v_t0 = kv_pool.tile([P, S], F32, name="v_t", tag="v")
        nc.sync.dma_start(v_t0[:], cv_d[0].rearrange("(p r) d -> p (r d)", p=P))

        # ---- prologue: constants + decode-row (key/value) correction terms
        ones_t = const_pool.tile([P, P], F32, name="ones_t")
        nc.vector.memset(ones_t[:], 1.0)
        ones_row = ones_t[0:1, :]
        ones_col = ones_t[:, 0:1]

        # -1 at partition pp, 0 elsewhere: masks out the stale cache row.
        piota = const_pool.tile([P, 1], F32, name="piota")
        nc.gpsimd.iota(
            piota[:], [[0, 1]], channel_multiplier=1,
            allow_small_or_imprecise_dtypes=True,
        )
        neg_e_pp = const_pool.tile([P, 1], F32, name="neg_e_pp")
        nc.vector.tensor_scalar(
            neg_e_pp[:],
            piota[:],
            float(pp),
            -1.0,
            op0=mybir.AluOpType.is_equal,
            op1=mybir.AluOpType.mult,
        )

        # q / key / value as [H, D] tiles (partition = head) on the ACT ring.
        q32 = const_pool.tile([H, D], F32, name="q32")
        nc.scalar.dma_start(q32[:], q_d.rearrange("h q d -> (h q) d"))
        key32 = const_pool.tile([H, D], F32, name="key32")
        nc.scalar.dma_start(key32[:], k_d.rearrange("h q d -> (h q) d"))
        value32 = const_pool.tile([H, D], F32, name="value32")
        nc.scalar.dma_start(value32[:], v_d.rearrange("h q d -> (h q) d"))

        # p_new[h] = exp(q_h . key_h * scale); coefs[:, h] = p_new[h] * e_h
        qk_prod = const_pool.tile([H, D], F32, name="qk_prod")
        nc.vector.tensor_tensor(qk_prod[:], q32[:], key32[:], op=mybir.AluOpType.mult)
        s_new = const_pool.tile([H, 1], F32, name="s_new")
        nc.vector.tensor_reduce(
            s_new[:], qk_prod[:], axis=mybir.AxisListType.X, op=mybir.AluOpType.add
        )
        p_new = const_pool.tile([H, 1], F32, name="p_new")
        nc.scalar.activation(
            p_new[:], s_new[:], mybir.ActivationFunctionType.Exp, scale=float(scale)
        )
        hmat = const_pool.tile([H, H], F32, name="hmat")
        nc.gpsimd.iota(
            hmat[:],
            [[-1, H]],
            channel_multiplier=1,
            allow_small_or_imprecise_dtypes=True,
        )
        coefs = const_pool.tile([H, H], F32, name="coefs")
        nc.vector.tensor_scalar(
            coefs[:],
            hmat[:],
            0.0,
            p_new[:],
            op0=mybir.AluOpType.is_equal,
            op1=mybir.AluOpType.mult,
        )

        out_stage = const_pool.tile([1, H * D], F32, name="out_stage")
        # out_stage doubles as the q staging row during the prologue (it is
        # only written by the per-head epilogues, which depend on q_bc).
        q_flat = out_stage
        nc.scalar.dma_start(q_flat[:], q_d.rearrange("h q d -> q (h d)"))
        q_bc = const_pool.tile([P, H * D], F32, name="q_bc")
        NB = 512
        for j in range((H * D + NB - 1) // NB):
            nb = min(NB, H * D - j * NB)
            qb_ps = ps_build.tile([P, NB], F32, name="qb_ps")
            nc.tensor.matmul(
                qb_ps[:, :nb],
                ones_row[:],
                q_flat[0:1, j * NB : j * NB + nb],
                start=True,
                stop=True,
            )
            # fold the 1/sqrt(D) softmax scale into the broadcast copy
            nc.scalar.mul(q_bc[:, j * NB : j * NB + nb], qb_ps[:, :nb], scale)

        mask = None
        if end_pos < S:
            # Additive score mask: 0 where s = p*R + r < end_pos, -1e30 after.
            s_iota = const_pool.tile([P, R], F32, name="s_iota")
            nc.gpsimd.iota(
                s_iota[:],
                [[1, R]],
                channel_multiplier=R,
                allow_small_or_imprecise_dtypes=True,
            )
            mask = const_pool.tile([P, R], F32, name="mask")
            nc.vector.tensor_scalar(
                mask[:],
                s_iota[:],
                float(end_pos),
                -1e30,
                op0=mybir.AluOpType.is_ge,
                op1=mybir.AluOpType.mult,
            )

        for h in range(H):
            # The last head's chain (mult -> reduce -> exp -> attn@V) is the
            # kernel's drain tail: split its stages in quarters so each stage
            # overlaps the tail of its K load and the chain after the last
            # HBM byte is only a quarter-chain.  Other heads stay whole
            # (splitting every load costs DMA descriptor efficiency).
            last = h == H - 1
            nsplit = 4 if last else 1
            RC, SC = R // nsplit, S // nsplit

            # All-fp32 numerics. K and V ride the SAME HWDGE (sync) ring:
            # per-ring FIFO makes each SDMA engine drain one load's 8x16KB
            # descriptors back-to-back (contiguous HBM bursts at line rate)
            # instead of ping-ponging between two rings' address streams at
            # 16KB granularity, which costs ~28% of HBM read efficiency.
            if h == 0:
                k_t, v_t = k_t0, v_t0
            else:
                k_t = kv_pool.tile([P, S], F32, name="k_t", tag="k")
                ck_h = ck_d[h].rearrange("(p r) d -> p (r d)", p=P)
                for c in range(nsplit):
                    nc.sync.dma_start(
                        k_t[:, c * SC : (c + 1) * SC], ck_h[:, c * SC : (c + 1) * SC]
                    )
                v_t = kv_pool.tile([P, S], F32, name="v_t", tag="v")
                cv_h = cv_d[h].rearrange("(p r) d -> p (r d)", p=P)
                vsplit = 2 if last else 1
                VC = S // vsplit
                for c in range(vsplit):
                    nc.sync.dma_start(
                        v_t[:, c * VC : (c + 1) * VC], cv_h[:, c * VC : (c + 1) * VC]
                    )

            # scores[p, r] = sum_d K[p, r, d] * q_scaled[d]   for s = p*R + r
            scores = sm_pool.tile([P, R], F32, name="scores", tag="scores")
            prod = sm_pool.tile([P, S], F32, name="prod", tag="prod", bufs=1)
            p_t = sm_pool.tile([P, R], F32, name="p_t", tag="p")
            z_cols = []
            for c in range(nsplit):
                qh = (
                    q_bc[:, h * D : (h + 1) * D]
                    .rearrange("p (o d) -> p o d", o=1)
                    .broadcast_to([P, RC, D])
                )
                k3 = k_t[:, c * SC : (c + 1) * SC].rearrange("p (r d) -> p r d", r=RC)
                prod3 = prod[:, c * SC : (c + 1) * SC].rearrange(
                    "p (r d) -> p r d", r=RC
                )
                sc_c = scores[:, c * RC : (c + 1) * RC]
                nc.vector.tensor_tensor(prod3, k3, qh, op=mybir.AluOpType.mult)
                nc.vector.tensor_reduce(
                    sc_c, prod3, axis=mybir.AxisListType.X, op=mybir.AluOpType.add
                )
                if mask is not None:
                    nc.vector.tensor_tensor(
                        sc_c,
                        sc_c,
                        mask[:, c * RC : (c + 1) * RC],
                        op=mybir.AluOpType.add,
                    )
                # p = exp(scores); z_col[p] = partial softmax denominator
                z_col = sm_pool.tile([P, 1], F32, name="z_col", tag=f"z{c}")
                nc.scalar.activation(
                    p_t[:, c * RC : (c + 1) * RC],
                    sc_c,
                    mybir.ActivationFunctionType.Exp,
                    accum_out=z_col[:],
                )
                z_cols.append(z_col)

            # tmp = -p_stale at partition pp (0 elsewhere): removes the stale
            # cache row's contribution from both attn@V and Z.
            tmp = sm_pool.tile([P, 1], F32, name="tmp", tag="tmp")
            nc.vector.tensor_tensor(
                tmp[:], p_t[:, rr : rr + 1], neg_e_pp[:], op=mybir.AluOpType.mult
            )

            # out_unnorm[1, D] = sum_r p[:, r]^T @ V_tile_r  (+ corrections)
            av_ps = ps_av.tile([1, D], F32, name="av_ps")
            for r in range(R):
                nc.tensor.matmul(
                    av_ps[:],
                    p_t[:, r : r + 1],
                    v_t[:, r * D : (r + 1) * D],
                    start=(r == 0),
                    stop=False,
                )
            nc.tensor.matmul(
                av_ps[:],
                tmp[:],
                v_t[:, rr * D : (rr + 1) * D],
                start=False,
                stop=False,
            )
            nc.tensor.matmul(
                av_ps[:],
                coefs[:, h : h + 1],
                value32[:],
                start=False,
                stop=True,
            )
            # Z = sum over partitions of the z_col partials (+ corrections)
            z_ps = ps_z.tile([1, 1], F32, name="z_ps")
            for c, z_col in enumerate(z_cols):
                nc.tensor.matmul(
                    z_ps[:], z_col[:], ones_col[:], start=(c == 0), stop=False
                )
            nc.tensor.matmul(z_ps[:], tmp[:], ones_col[:], start=False, stop=False)
            nc.tensor.matmul(
                z_ps[:], coefs[:, h : h + 1], ones_col[0:H, :], start=False, stop=True
            )
            rz = sm_pool.tile([1, 1], F32, name="rz", tag="rz")
            nc.vector.reciprocal(rz[:], z_ps[:])
            nc.scalar.mul(out_stage[0:1, h * D : (h + 1) * D], av_ps[:], rz[:])
            if h == H - 2:
                # everything but the last head is final: ship it early so the
                # drain only carries the last head's 512B.
                nc.scalar.dma_start(
                    out_d[0:1, : (H - 1) * D], out_stage[0:1, : (H - 1) * D]
                )

        nc.scalar.dma_start(
            out_d[0:1, (H - 1) * D :], out_stage[0:1, (H - 1) * D :]
        )

    nc.compile()
    return nc


def _get_program(H, S, D, cache_pos):
    key = (H, S, D, cache_pos)
    if key not in _program_cache:
        _program_cache[key] = _build(H, S, D, cache_pos)
    return _program_cache[key]


def kernel(query, key, value, cache_k, cache_v, cache_pos):
    cache_pos = int(cache_pos)
    B, H, Q, D = query.shape
    S = cache_k.shape[2]
    assert Q == 1 and B == N_CORES

    nc = _get_program(H, S, D, cache_pos)

    f32 = np.float32
    in_maps = [
        {
            "query": np.ascontiguousarray(query[b], dtype=f32),
            "key": np.ascontiguousarray(key[b], dtype=f32),
            "value": np.ascontiguousarray(value[b], dtype=f32),
            "cache_k": np.ascontiguousarray(cache_k[b], dtype=f32),
            "cache_v": np.ascontiguousarray(cache_v[b], dtype=f32),
        }
        for b in range(B)
    ]
    res = run_bass_kernel_spmd(nc, in_maps, core_ids=list(range(N_CORES)))
    global _last_results
    _last_results = res
    out = np.stack(
        [res.results[b]["out"].reshape(H, 1, D).astype(np.float32) for b in range(B)]
    )
    return out


# revision 6
# speedup vs baseline: 1.3636x; 1.1745x over previous
"""Cached scaled-dot-product-attention decode kernel for Trainium2 (Bass/Tile).

Full inputs -> shard batch across 8 NeuronCores (B=8, one batch per core)
-> per-core Bass kernel computes, for each of its 32 heads:
    K = cache_k[h] with row cache_pos replaced by key[h]
    V = cache_v[h] with row cache_pos replaced by value[h]
    out[h] = softmax(q K^T / sqrt(D)) V        (over the first cache_pos+1 rows)
-> gather per-core outputs into the full [B, H, 1, D] array.

Layout trick: cache_k[h] ([S, D] row-major in HBM) is loaded as SBUF
[128, S] via "(p r) d -> p (r d)" so every partition reads one fully
contiguous 16KB chunk (max DMA efficiency).  Sequence position
s = p*R + r lands at (partition p, column-block r).  This is a fixed
permutation of the sequence axis, which softmax(..)V is invariant to, as
long as K and V use the same permutation (they do).

The kernel is HBM-bandwidth-bound (128 MiB of cache per core vs the
~358 GB/s HBM-per-NeuronCore limit -> ~375 us floor); the design keeps
one clean DMA stream and minimizes the fill/drain overhead around it:

 - K and V ride the SAME HWDGE (sync) ring.  Per-ring FIFO makes each
   SDMA engine drain one load's 8x16KB descriptors back-to-back
   (contiguous HBM bursts at line rate).  Two free-running rings would
   ping-pong between the two address streams at 16KB granularity and
   cost ~28% of HBM read efficiency (measured).
 - No cache-row scatter DMAs.  The decode-step key/value are NOT written
   into the loaded tiles (that would serialize each load's descriptor
   generation on the previous load's completion receipt).  The stale
   cache row's contribution is removed and the new row's added
   algebraically in the PSUM accumulation:
       out_unnorm += p_new * value[h] - p_stale * V_cache[pos]
       Z          += p_new           - p_stale
   via two extra rank-1 matmuls per head, where p_stale falls out of the
   normal score pipeline and p_new = exp(q . key * scale) is precomputed
   for all heads in the prologue.
 - kv tile pools are 4-deep so a load's WAR dependency (4 heads back) is
   always long resolved; load triggers never gate the descriptor ring.
 - attn@V runs in bf16 on the PE (1 cycle/row vs fp32's 4): V tiles are
   cast fp32->bf16 on the otherwise-idle ACT engine and exp writes bf16
   probabilities (softmax denominators still accumulate in fp32, scores
   are computed entirely in fp32 — only the final weighted average is
   bf16, ~0.4% element error vs the 2e-2 tolerance).
 - The last head is split 4-ways (K load, mult, reduce, exp, V load,
   V cast, attn@V all chunked) and its K is hoisted ahead of head H-2's
   V in the ring, so the drain after the last HBM byte is only a
   quarter-chain; the output for heads 0..H-2 is shipped early.

Scores are computed on the DVE (one big elementwise multiply against a
partition-broadcast q, then a 3D tensor_reduce over d) so K never needs
a transpose; DVE only runs tensor_tensor / tensor_reduce (+ [1,1]
reciprocals), which never contend for the shared SBUF port pair.  exp is
unshifted — scores are ~N(0,1) so fp32 exp cannot overflow.
"""

import math
from contextlib import ExitStack

import numpy as np

import concourse.bacc as bacc
import concourse.mybir as mybir
import concourse.tile as tile
from concourse.bass_utils import run_bass_kernel_spmd

F32 = mybir.dt.float32
BF16 = mybir.dt.bfloat16

N_CORES = 8

_program_cache: dict = {}
_last_results = None


def _build(H: int, S: int, D: int, cache_pos: int):
    """Build + compile the per-core Bass program (identical on all cores)."""
    P = 128
    R = S // P  # column blocks / rows-per-partition (32 for S=4096)
    assert S % P == 0 and D == 128
    end_pos = cache_pos + 1
    scale = 1.0 / math.sqrt(D)

    nc = bacc.Bacc(
        "TRN2",
        target_bir_lowering=False,
        debug=False,
        enable_asserts=False,
        num_devices=N_CORES,
    )
    q_d = nc.dram_tensor("query", [H, 1, D], F32, kind="ExternalInput").ap()
    k_d = nc.dram_tensor("key", [H, 1, D], F32, kind="ExternalInput").ap()
    v_d = nc.dram_tensor("value", [H, 1, D], F32, kind="ExternalInput").ap()
    ck_d = nc.dram_tensor("cache_k", [H, S, D], F32, kind="ExternalInput").ap()
    cv_d = nc.dram_tensor("cache_v", [H, S, D], F32, kind="ExternalInput").ap()
    out_d = nc.dram_tensor("out", [1, H * D], F32, kind="ExternalOutput").ap()

    pp = cache_pos // R  # partition holding the patched row
    rr = cache_pos % R  # column block holding the patched row

    with tile.TileContext(nc) as tc, ExitStack() as ctx:
        const_pool = ctx.enter_context(tc.tile_pool(name="const", bufs=1))
        kv_pool = ctx.enter_context(tc.tile_pool(name="kv", bufs=4))
        vbf_pool = ctx.enter_context(tc.tile_pool(name="vbf", bufs=2))
        sm_pool = ctx.enter_context(tc.tile_pool(name="sm", bufs=2))
        ps_build = ctx.enter_context(tc.tile_pool(name="psb", bufs=2, space="PSUM"))
        ps_av = ctx.enter_context(tc.tile_pool(name="psav", bufs=2, space="PSUM"))
        ps_z = ctx.enter_context(tc.tile_pool(name="psz", bufs=2, space="PSUM"))

        # ---- head 0's big loads go first so the DMA ring starts instantly
        k_t0 = kv_pool.tile([P, S], F32, name="k_t", tag="k")
        nc.sync.dma_start(k_t0[:], ck_d[0].rearrange("(p r) d -> p (r d)", p=P))
        v_t0 = kv_pool.tile([P, S], F32, name="v_t", tag="v")
        nc.sync.dma_start(v_t0[:], cv_d[0].rearrange("(p r) d -> p (r d)", p=P))

        # ---- prologue: constants + decode-row (key/value) correction terms
        ones_t = const_pool.tile([P, P], F32, name="ones_t")
        nc.vector.memset(ones_t[:], 1.0)
        ones_row = ones_t[0:1, :]
        ones_col = ones_t[:, 0:1]
        ones_bf = const_pool.tile([P, 1], BF16, name="ones_bf")
        nc.vector.memset(ones_bf[:], 1.0)

        # -1 at partition pp, 0 elsewhere: masks out the stale cache row.
        piota = const_pool.tile([P, 1], F32, name="piota")
        nc.gpsimd.iota(
            piota[:], [[0, 1]], channel_multiplier=1,
            allow_small_or_imprecise_dtypes=True,
        )
        neg_e_pp = const_pool.tile([P, 1], BF16, name="neg_e_pp")
        nc.vector.tensor_scalar(
            neg_e_pp[:],
            piota[:],
            float(pp),
            -1.0,
            op0=mybir.AluOpType.is_equal,
            op1=mybir.AluOpType.mult,
        )

        # q / key / value as [H, D] tiles (partition = head) on the ACT ring.
        q32 = const_pool.tile([H, D], F32, name="q32")
        nc.scalar.dma_start(q32[:], q_d.rearrange("h q d -> (h q) d"))
        key32 = const_pool.tile([H, D], F32, name="key32")
        nc.scalar.dma_start(key32[:], k_d.rearrange("h q d -> (h q) d"))
        value32 = const_pool.tile([H, D], F32, name="value32")
        nc.scalar.dma_start(value32[:], v_d.rearrange("h q d -> (h q) d"))
        value_bf = const_pool.tile([H, D], BF16, name="value_bf")
        nc.scalar.copy(value_bf[:], value32[:])

        # p_new[h] = exp(q_h . key_h * scale); coefs[:, h] = p_new[h] * e_h
        qk_prod = const_pool.tile([H, D], F32, name="qk_prod")
        nc.vector.tensor_tensor(qk_prod[:], q32[:], key32[:], op=mybir.AluOpType.mult)
        s_new = const_pool.tile([H, 1], F32, name="s_new")
        nc.vector.tensor_reduce(
            s_new[:], qk_prod[:], axis=mybir.AxisListType.X, op=mybir.AluOpType.add
        )
        p_new = const_pool.tile([H, 1], F32, name="p_new")
        nc.scalar.activation(
            p_new[:], s_new[:], mybir.ActivationFunctionType.Exp, scale=float(scale)
        )
        hmat = const_pool.tile([H, H], F32, name="hmat")
        nc.gpsimd.iota(
            hmat[:],
            [[-1, H]],
            channel_multiplier=1,
            allow_small_or_imprecise_dtypes=True,
        )
        coefs = const_pool.tile([H, H], BF16, name="coefs")
        nc.vector.tensor_scalar(
            coefs[:],
            hmat[:],
            0.0,
            p_new[:],
            op0=mybir.AluOpType.is_equal,
            op1=mybir.AluOpType.mult,
        )

        out_stage = const_pool.tile([1, H * D], F32, name="out_stage")
        # out_stage doubles as the q staging row during the prologue (it is
        # only written by the per-head epilogues, which depend on q_bc).
        q_flat = out_stage
        nc.scalar.dma_start(q_flat[:], q_d.rearrange("h q d -> q (h d)"))
        q_bc = const_pool.tile([P, H * D], F32, name="q_bc")
        NB = 512
        for j in range((H * D + NB - 1) // NB):
            nb = min(NB, H * D - j * NB)
            qb_ps = ps_build.tile([P, NB], F32, name="qb_ps")
            nc.tensor.matmul(
                qb_ps[:, :nb],
                ones_row[:],
                q_flat[0:1, j * NB : j * NB + nb],
                start=True,
                stop=True,
            )
            # fold the 1/sqrt(D) softmax scale into the broadcast copy
            nc.scalar.mul(q_bc[:, j * NB : j * NB + nb], qb_ps[:, :nb], scale)

        mask = None
        if end_pos < S:
            # Additive score mask: 0 where s = p*R + r < end_pos, -1e30 after.
            s_iota = const_pool.tile([P, R], F32, name="s_iota")
            nc.gpsimd.iota(
                s_iota[:],
                [[1, R]],
                channel_multiplier=R,
                allow_small_or_imprecise_dtypes=True,
            )
            mask = const_pool.tile([P, R], F32, name="mask")
            nc.vector.tensor_scalar(
                mask[:],
                s_iota[:],
                float(end_pos),
                -1e30,
                op0=mybir.AluOpType.is_ge,
                op1=mybir.AluOpType.mult,
            )

        prefetched_k = {}
        for h in range(H):
            # The last head's chain (mult -> reduce -> exp -> attn@V) is the
            # kernel's drain tail: split its stages in quarters so each stage
            # overlaps the tail of its K load and the chain after the last
            # HBM byte is only a quarter-chain.  Other heads stay whole
            # (splitting every load costs DMA descriptor efficiency).
            last = h == H - 1
            nsplit = 4 if last else 1
            RC, SC = R // nsplit, S // nsplit

            if h == 0:
                k_t, v_t = k_t0, v_t0
            else:
                if h in prefetched_k:
                    k_t = prefetched_k.pop(h)
                else:
                    k_t = kv_pool.tile([P, S], F32, name="k_t", tag="k")
                    ck_h = ck_d[h].rearrange("(p r) d -> p (r d)", p=P)
                    for c in range(nsplit):
                        nc.sync.dma_start(
                            k_t[:, c * SC : (c + 1) * SC],
                            ck_h[:, c * SC : (c + 1) * SC],
                        )
                if h == H - 2 and H >= 3:
                    # hoist the last head's K ahead of this head's V in the
                    # ring so the final DVE score chain starts ~6us earlier.
                    kn = kv_pool.tile([P, S], F32, name="k_t", tag="k")
                    ck_n = ck_d[h + 1].rearrange("(p r) d -> p (r d)", p=P)
                    QC = S // 4
                    for c in range(4):
                        nc.sync.dma_start(
                            kn[:, c * QC : (c + 1) * QC],
                            ck_n[:, c * QC : (c + 1) * QC],
                        )
                    prefetched_k[h + 1] = kn
                v_t = kv_pool.tile([P, S], F32, name="v_t", tag="v")
                cv_h = cv_d[h].rearrange("(p r) d -> p (r d)", p=P)
                vsplit = 4 if last else 1
                VC = S // vsplit
                for c in range(vsplit):
                    nc.sync.dma_start(
                        v_t[:, c * VC : (c + 1) * VC], cv_h[:, c * VC : (c + 1) * VC]
                    )

            # V in bf16 for the PE (cast on ACT, which is otherwise idle)
            v_bf = vbf_pool.tile([P, S], BF16, name="v_bf", tag="vbf")
            vcsplit = 4 if last else 1
            VCC = S // vcsplit
            for c in range(vcsplit):
                nc.scalar.copy(
                    v_bf[:, c * VCC : (c + 1) * VCC], v_t[:, c * VCC : (c + 1) * VCC]
                )

            # scores[p, r] = sum_d K[p, r, d] * q_scaled[d]   for s = p*R + r
            scores = sm_pool.tile([P, R], F32, name="scores", tag="scores")
            prod = sm_pool.tile([P, S], F32, name="prod", tag="prod", bufs=1)
            p_t = sm_pool.tile([P, R], BF16, name="p_t", tag="p")
            z_cols = []
            for c in range(nsplit):
                qh = (
                    q_bc[:, h * D : (h + 1) * D]
                    .rearrange("p (o d) -> p o d", o=1)
                    .broadcast_to([P, RC, D])
                )
                k3 = k_t[:, c * SC : (c + 1) * SC].rearrange("p (r d) -> p r d", r=RC)
                prod3 = prod[:, c * SC : (c + 1) * SC].rearrange(
                    "p (r d) -> p r d", r=RC
                )
                sc_c = scores[:, c * RC : (c + 1) * RC]
                nc.vector.tensor_tensor(prod3, k3, qh, op=mybir.AluOpType.mult)
                nc.vector.tensor_reduce(
                    sc_c, prod3, axis=mybir.AxisListType.X, op=mybir.AluOpType.add
                )
                if mask is not None:
                    nc.vector.tensor_tensor(
                        sc_c,
                        sc_c,
                        mask[:, c * RC : (c + 1) * RC],
                        op=mybir.AluOpType.add,
                    )
                # p = exp(scores) in bf16; z_col[p] = fp32 partial denominator
                z_col = sm_pool.tile([P, 1], F32, name="z_col", tag=f"z{c}")
                nc.scalar.activation(
                    p_t[:, c * RC : (c + 1) * RC],
                    sc_c,
                    mybir.ActivationFunctionType.Exp,
                    accum_out=z_col[:],
                )
                z_cols.append(z_col)

            # tmp = -p_stale at partition pp (0 elsewhere): removes the stale
            # cache row's contribution from both attn@V and Z.
            tmp = sm_pool.tile([P, 1], BF16, name="tmp", tag="tmp")
            nc.vector.tensor_tensor(
                tmp[:], p_t[:, rr : rr + 1], neg_e_pp[:], op=mybir.AluOpType.mult
            )

            # out_unnorm[1, D] = sum_r p[:, r]^T @ V_tile_r  (+ corrections)
            av_ps = ps_av.tile([1, D], F32, name="av_ps")
            for r in range(R):
                nc.tensor.matmul(
                    av_ps[:],
                    p_t[:, r : r + 1],
                    v_bf[:, r * D : (r + 1) * D],
                    start=(r == 0),
                    stop=False,
                )
            nc.tensor.matmul(
                av_ps[:],
                tmp[:],
                v_bf[:, rr * D : (rr + 1) * D],
                start=False,
                stop=False,
            )
            nc.tensor.matmul(
                av_ps[:],
                coefs[:, h : h + 1],
                value_bf[:],
                start=False,
                stop=True,
            )
            # Z = sum over partitions of the z_col partials (+ corrections)
            z_ps = ps_z.tile([1, 1], F32, name="z_ps")
            for c, z_col in enumerate(z_cols):
                nc.tensor.matmul(
                    z_ps[:], z_col[:], ones_col[:], start=(c == 0), stop=False
                )
            nc.tensor.matmul(z_ps[:], tmp[:], ones_bf[:], start=False, stop=False)
            nc.tensor.matmul(
                z_ps[:], coefs[:, h : h + 1], ones_bf[0:H, :], start=False, stop=True
            )
            rz = sm_pool.tile([1, 1], F32, name="rz", tag="rz")
            nc.vector.reciprocal(rz[:], z_ps[:])
            nc.scalar.mul(out_stage[0:1, h * D : (h + 1) * D], av_ps[:], rz[:])
            if h == H - 2:
                # everything but the last head is final: ship it early so the
                # drain only carries the last head's 512B.
                nc.scalar.dma_start(
                    out_d[0:1, : (H - 1) * D], out_stage[0:1, : (H - 1) * D]
                )

        nc.scalar.dma_start(
            out_d[0:1, (H - 1) * D :], out_stage[0:1, (H - 1) * D :]
        )

    nc.compile()
    return nc


def _get_program(H, S, D, cache_pos):
    key = (H, S, D, cache_pos)
    if key not in _program_cache:
        _program_cache[key] = _build(H, S, D, cache_pos)
    return _program_cache[key]


def kernel(query, key, value, cache_k, cache_v, cache_pos):
    cache_pos = int(cache_pos)
    B, H, Q, D = query.shape
    S = cache_k.shape[2]
    assert Q == 1 and B == N_CORES

    nc = _get_program(H, S, D, cache_pos)

    f32 = np.float32
    in_maps = [
        {
            "query": np.ascontiguousarray(query[b], dtype=f32),
            "key": np.ascontiguousarray(key[b], dtype=f32),
            "value": np.ascontiguousarray(value[b], dtype=f32),
            "cache_k": np.ascontiguousarray(cache_k[b], dtype=f32),
            "cache_v": np.ascontiguousarray(cache_v[b], dtype=f32),
        }
        for b in range(B)
    ]
    res = run_bass_kernel_spmd(nc, in_maps, core_ids=list(range(N_CORES)))
    global _last_results
    _last_results = res
    out = np.stack(
        [res.results[b]["out"].reshape(H, 1, D).astype(np.float32) for b in range(B)]
    )
    return out
